# revision 55
# baseline (speedup 1.0000x reference)
# Relational GCN message-passing layer (MolGAN-style) on 8 Trainium2 NeuronCores.
#
#   x_new[s,i,b] = tanh( sum_c norm[s,i,c] * sum_{j,a} A[s,i,j,c] x[s,j,a] W[a,b,c]
#                        + (x @ theta_root)[s,i,b] )
#   norm[s,i,c] = 1 / (sum_j A[s,i,j,c] + eps)        (c < 4; channel 4 dropped)
#
# Sharding: data-parallel over the batch dim s — 16 batches / 8 cores = 2 per core.
#
# v2 dataflow — host-side layout prep shrinks the HBM stream 2.5x:
#   kernel() drops the unused 5th relation channel and casts A to fp16 ON HOST
#   (same information loss as the baseline's in-flight DMA fp32->fp16 cast),
#   and pre-permutes it to AT[s, ib, jb, j(128), c, i(128)] so each [j, i]
#   128x128 tile lands in SBUF already in matmul-lhsT orientation. The device
#   kernel then:
#   1. streams one (s, ib) "chunk" = 8 jb-tiles x 4 c = 1.05 MB per DMA,
#      contiguous 1 KB runs per (jb, partition) — full DMA-engine speed;
#   2. stage-1 per c: m[i, 0:129] = sum_jb AT(jb,c).T @ x~[jb], where x~ has a
#      ones column -> col 128 is the degree row-sum (normalizer) for free;
#   3. norm = 1/rowsum (DVE reciprocal), applied as the per-partition scale of
#      the ACT PSUM->SBUF copy (out = psum * norm, cast to fp16);
#   4. m tiles transposed back (PE) so stage-2 contracts over (c,a):
#      out[i,b] = sum_c mT_c.T @ W_c + xT.T @ theta  (5 accumulating matmuls);
#   5. tanh on ACT (PSUM -> SBUF fp32), batched HWDGE DMA out.
#   No PE transposes of A remain (the baseline spent ~40% of PE on them), so
#   the PE runs ~2.3us per 2.9us chunk DMA and the kernel sits on the DMA
#   roofline at 2.5x fewer bytes than the fp32 stream.

import os
from contextlib import ExitStack

import numpy as np

import concourse.tile as tile
from concourse import bacc, mybir
from concourse.bass_utils import run_bass_kernel_spmd
from concourse.masks import make_identity

S, N, R, CIN, COUT = 16, 1024, 4, 128, 128
NCORES = 8
SPC = S // NCORES  # batches per core
NB = N // 128      # 128-row node blocks
XW = CIN + 2       # x~ row stride: 128 data + 1 ones + 1 pad (4B alignment)

F16 = mybir.dt.float16
F32 = mybir.dt.float32
F8 = mybir.dt.float8e4


def _kernel_body(
    tc,
    bench_iters=1,
    chunk_bufs=6,
    chunk_first=2,
    n_sub=1,
    y_batch=4,
    small_bufs=4,
    pm_bufs=2,
    pmt_bufs=2,
    po_bufs=2,
    dma_off=False,
    stages="all",
    y_stores=True,
    dma_eng="sync",
    y_eng="scalar",
    pipe=1,
    a_dt="f8",
    dr=1,
    y_dt="f16",
    flow="xw",
    s1_order="mdeg",
    dbg_skip=(),
):
    nc = tc.nc
    AD = F8 if a_dt == "f8" else F16
    YD = F16 if y_dt == "f16" else F32
    if flow == "xw":
        assert a_dt == "f8" and dr, "xw flow is fp8-DoubleRow only"
    # Host-prepped A: [s, ib, j(128), jb, c, i(128)], channel 4 dropped, cast
    # to fp8e4/fp16 on host. j (the partition dim) is OUTER so each
    # partition's chunk slice is one contiguous run -> 128 DMA descriptors
    # per chunk instead of 1024.
    A = nc.dram_tensor("A", (SPC, NB, 128, NB, R, 128), AD, kind="ExternalInput").ap()
    w = nc.dram_tensor("weight", (CIN, COUT, R), F32, kind="ExternalInput").ap()
    th = nc.dram_tensor("theta_root", (CIN, COUT), F32, kind="ExternalInput").ap()
    y = nc.dram_tensor("y", (SPC, N, COUT), YD, kind="ExternalOutput").ap()
    if flow == "xw":
        # host-prepped: x cast to fp8 [s, j, a] (stage-1 weights) and x^T in
        # fp16 [s, a, i] (theta-term lhsT) — no on-device transposes/casts
        x8 = nc.dram_tensor("x8", (SPC, N, CIN), F8, kind="ExternalInput").ap()
        xTh = nc.dram_tensor("xT", (SPC, CIN, N), F16, kind="ExternalInput").ap()
    else:
        x = nc.dram_tensor("x", (SPC, N, CIN), F16, kind="ExternalInput").ap()

    with ExitStack() as ctx:
        consts = ctx.enter_context(tc.tile_pool(name="consts", bufs=1))
        chunks = ctx.enter_context(tc.tile_pool(name="chunks", bufs=chunk_bufs))
        small = ctx.enter_context(tc.tile_pool(name="small", bufs=small_bufs))
        outp = ctx.enter_context(tc.tile_pool(name="outp", bufs=2))
        # PSUM is 8 banks; pools are bank-granular per (tag, buf).
        if flow == "xw":
            pmt2 = ctx.enter_context(tc.tile_pool(name="pmt2", bufs=2, space="PSUM"))
            pdeg = ctx.enter_context(tc.tile_pool(name="pdeg", bufs=2, space="PSUM"))
            po = ctx.enter_context(tc.tile_pool(name="po", bufs=2, space="PSUM"))
        else:
            pm = ctx.enter_context(
                tc.tile_pool(name="pm", bufs=pm_bufs, space="PSUM")
            )
            pmt = ctx.enter_context(
                tc.tile_pool(name="pmt", bufs=pmt_bufs, space="PSUM")
            )
            po = ctx.enter_context(tc.tile_pool(name="po", bufs=po_bufs, space="PSUM"))

        engs = {
            "gpsimd": nc.gpsimd,
            "sync": nc.sync,
            "scalar": nc.scalar,
            "vector": nc.vector,
        }
        st_eng = engs[y_eng]

        def load_chunk(si, ib):
            # "alt": alternate chunks between the two HWDGE queues (SP/ACT) so
            # one queue's AP-walk/setup overlaps the other's transfer.
            if dma_eng == "alt":
                ld_eng = engs["sync"] if (si * NB + ib) % 2 == 0 else engs["scalar"]
            else:
                ld_eng = engs[dma_eng]
            # 8 jb-tiles [128j, 4c, 128i]; HWDGE (sync/scalar/vector) walks the
            # AP in hardware — no per-descriptor SWDGE generation time. n_sub
            # sub-DMAs let the first stage-1 matmuls start before the whole
            # chunk lands.
            t = chunks.tile([128, NB, R, 128], AD, tag="chunk")
            if dma_off:
                # memset on the otherwise-idle GPSIMD queue so the fake
                # producer dependency doesn't serialize through DVE/ACT
                nc.gpsimd.memset(t[:, :1, :1, :1], 0.5)
                return t
            step = NB // n_sub
            for q in range(n_sub):
                ld_eng.dma_start(
                    out=t[:, q * step : (q + 1) * step, :, :],
                    in_=A[si, ib, :, q * step : (q + 1) * step, :, :],
                )
            return t

        # Kick off the A stream before the (Pool-queue) prelude loads so HBM
        # isn't idle during the first few us. Only meaningful single-shot.
        preloaded = []
        if bench_iters == 1:
            for t in range(chunk_first):
                preloaded.append(load_chunk(*divmod(t, NB)))

        if flow != "xw":
            ident = consts.tile([128, 128], F16)
            make_identity(nc, ident)

        # weight [a,b,c] -> w2 [a,c,b] fp16 so stage-2 rhs streams contiguously
        wtmp = consts.tile([128, COUT * R], F16)
        nc.gpsimd.dma_start(out=wtmp, in_=w.rearrange("a b c -> a (b c)"))
        w2 = consts.tile([128, R, COUT], F16)
        wv = wtmp.rearrange("a (b c) -> a b c", c=R)
        for c in range(R):
            nc.vector.tensor_copy(out=w2[:, c, :], in_=wv[:, :, c])
        th16 = consts.tile([128, COUT], F16)
        nc.gpsimd.dma_start(out=th16, in_=th)

        if flow == "xw":
            # x8 tiles [j, a] fp8 — stage-1 stationary weights. Ldweights
            # needs 4 B-aligned k-tile strides: 128 B rows.
            xw8 = consts.tile([128, SPC * NB, CIN], F8)
            nc.sync.dma_start(
                out=xw8, in_=x8.rearrange("s (jb p) a -> p (s jb) a", p=128)
            )
            # xT tiles [a, i] fp16 for the theta_root term (the theta path
            # dominates the output magnitude, keep its precision)
            xT = consts.tile([128, SPC * NB, CIN], F16)
            nc.sync.dma_start(
                out=xT.rearrange("p (s t) i -> p s t i", s=SPC),
                in_=xTh.rearrange("s a (t p) -> a s t p", p=128),
            )
            # all-ones M=128 lhsT for the degree matmul: the degree row comes
            # out replicated on every PSUM partition, which is exactly the
            # broadcast the norm-multiply needs (M=1/M=2 ldweights fail the
            # neuronxcc ISA check anyway)
            ones8 = consts.tile([128, 2, CIN], F8)
            nc.vector.memset(ones8, 1.0)
        else:
            # x~ tiles: [j, 0:128]=x (fp16), col 128 = 1.0 (rowsum probe)
            xe = consts.tile([128, SPC * NB, XW], F16)
            nc.vector.memset(xe[:, :, CIN], 1.0)
            nc.gpsimd.dma_start(
                out=xe[:, :, :CIN],
                in_=x.rearrange("s (jb p) a -> p (s jb) a", p=128),
            )
            # xT tiles [a, i] for the theta_root term
            xT = consts.tile([128, SPC * NB, CIN], F16)
            for k in range(SPC * NB):
                pt = pmt.tile([128, 128], F16, tag="mt")
                nc.tensor.transpose(pt, xe[:, k, :CIN], ident)
                nc.vector.tensor_copy(out=xT[:, k, :], in_=pt)
            if dr:
                xe8 = consts.tile([128, SPC * NB, XW], F8)
                nc.vector.tensor_copy(out=xe8, in_=xe)

        def stage1_mm(si, chunk_t):
            # m_c[i, 0:129] = sum_jb AT(jb,c).T @ x~[jb]; col 128 = degree
            # rowsum. Two PSUM tiles of 2 c-planes each (a [128,4,129] fp32
            # tile would be 2064 B/partition — 16 B over a PSUM bank).
            m01 = pm.tile([128, 2, CIN + 1], F32, tag="m01")
            m23 = pm.tile([128, 2, CIN + 1], F32, tag="m23")
            for c in range(R):
                m = (m01, m23)[c // 2][:, c % 2, :]
                if dr:
                    # fp8 DoubleRow: K=256 per matmul (2 jb-tiles as k-tiles
                    # in dim 1 of both APs) at 2 PSUM rows/cycle.
                    for q in range(NB // 2):
                        k = si * NB + 2 * q
                        nc.tensor.matmul(
                            m,
                            lhsT=chunk_t[:, 2 * q : 2 * q + 2, c, :],
                            rhs=xe8[:, k : k + 2, : CIN + 1],
                            start=(q == 0),
                            stop=(q == NB // 2 - 1),
                            perf_mode=mybir.MatmulPerfMode.DoubleRow,
                        )
                else:
                    for jb in range(NB):
                        nc.tensor.matmul(
                            m,
                            lhsT=chunk_t[:, jb, c, :],
                            rhs=xe[:, si * NB + jb, : CIN + 1],
                            start=(jb == 0),
                            stop=(jb == NB - 1),
                        )
            return m01, m23

        def stage1_norm(c, m01, m23):
            # norm = 1/rowsum (DVE), applied as the per-partition scale of the
            # ACT PSUM->SBUF copy, then PE-transposed for the stage-2
            # (c,a)-contraction. Runs one software-pipeline step behind
            # stage1_mm so the PE transpose never stalls the PE queue waiting
            # on the DVE->ACT chain.
            m = (m01, m23)[c // 2][:, c % 2, :]
            nrm = small.tile([128, 1], F32, tag="norm")
            nc.vector.reciprocal(nrm, m[:, CIN : CIN + 1])
            mn = small.tile([128, CIN], F16, tag="mn")
            nc.scalar.mul(mn, m[:, :CIN], nrm)  # psum * norm -> fp16 SBUF
            pt = pmt.tile([128, 128], F16, tag="mt")
            nc.tensor.transpose(pt, mn, ident)
            mt = small.tile([128, CIN], F16, tag="mts")
            nc.vector.tensor_copy(out=mt, in_=pt)
            return mt

        yb = {"tile": None}
        yv = y.rearrange("s (t p) b -> p (s t) b", p=128)

        def stage2(si, ib, mts):
            out_ps = po.tile([128, COUT], F32, tag="o")
            for c in range(R):
                nc.tensor.matmul(
                    out_ps, lhsT=mts[c], rhs=w2[:, c, :], start=(c == 0), stop=False
                )
            nc.tensor.matmul(
                out_ps, lhsT=xT[:, si * NB + ib, :], rhs=th16, start=False, stop=True
            )
            t = si * NB + ib
            k = t % y_batch
            if k == 0:
                yacc = outp.tile([128, y_batch, COUT], YD, tag="yacc")
                yb["tile"] = yacc
            nc.scalar.activation(
                yb["tile"][:, k, :], out_ps, mybir.ActivationFunctionType.Tanh
            )
            if k == y_batch - 1 and y_stores:
                t0 = t - k
                st_eng.dma_start(out=yv[:, t0 : t0 + y_batch, :], in_=yb["tile"])

        def finish(si, ib, m01, m23):
            mts = [stage1_norm(c, m01, m23) for c in range(R)]
            stage2(si, ib, mts)

        # --- x-as-weights flow: x~ tiles are the PE-stationary operand (4
        # weight loads per chunk instead of 16), A streams as the moving
        # tensor, stage-1 output lands directly in the transposed [a, (c,i)]
        # orientation stage-2 wants, and the degree row rides an extra
        # ones-lhsT matmul (re-streams the chunk at 0.5 cyc/col, no weight
        # cost). Norm is broadcast to all partitions by a K=1 matmul and
        # applied with one DVE multiply.
        def stage1_mm_xw(si, chunk_t):
            mT = pmt2.tile([128, R, 128], F32, tag="mT")  # [a, (c, i)]
            deg = pdeg.tile([128, R * 128], F32, tag="deg")  # [(c, i)] x128
            nq = NB // 2

            def mm(q):
                nc.tensor.matmul(
                    mT,
                    lhsT=xw8[:, si * NB + 2 * q : si * NB + 2 * q + 2, :],
                    rhs=chunk_t[:, 2 * q : 2 * q + 2, :, :],
                    start=(q == 0),
                    stop=(q == nq - 1),
                    perf_mode=mybir.MatmulPerfMode.DoubleRow,
                )

            def dm(q):
                nc.tensor.matmul(
                    deg,
                    lhsT=ones8,
                    rhs=chunk_t[:, 2 * q : 2 * q + 2, :, :],
                    start=(q == 0),
                    stop=(q == nq - 1),
                    perf_mode=mybir.MatmulPerfMode.DoubleRow,
                )

            skip_deg = "deg" in dbg_skip
            if skip_deg:
                nc.vector.memset(deg, 1.0)
            if s1_order == "mdeg":
                # alternate the two PSUM banks so one matmul's access-latency
                # tail hides under the other's stream
                for q in range(nq):
                    mm(q)
                    if not skip_deg:
                        dm(q)
            else:
                for q in range(nq):
                    mm(q)
                if not skip_deg:
                    for q in range(nq):
                        dm(q)
            return mT, deg

        def norm_apply_xw(mT, deg):
            # deg is already replicated across partitions; reciprocal it to
            # SBUF fp16, then one DVE multiply applies the norm and casts the
            # stage-2 lhsT to fp16. (TensorTensor may read only one PSUM
            # operand — nrmb is the SBUF one.)
            nrmb = small.tile([128, R * 128], F32, tag="nrmb")
            # ~18-bit approx reciprocal, ~5x faster on DVE than the exact op
            # (norm only needs ~8 bits; deg ~512 so no denorm/inf edge cases)
            nc.vector.reciprocal_approx_fast(nrmb, deg)
            mts = small.tile([128, R, 128], F16, tag="mts")
            nc.vector.tensor_mul(mts, mT, nrmb)
            return mts

        def stage2_xw(si, ib, mts):
            out_ps = po.tile([128, COUT], F32, tag="o")
            for c in range(R):
                nc.tensor.matmul(
                    out_ps,
                    lhsT=mts[:, c, :],
                    rhs=w2[:, c, :],
                    start=(c == 0),
                    stop=False,
                )
            nc.tensor.matmul(
                out_ps, lhsT=xT[:, si * NB + ib, :], rhs=th16, start=False, stop=True
            )
            t = si * NB + ib
            k = t % y_batch
            if k == 0:
                yacc = outp.tile([128, y_batch, COUT], YD, tag="yacc")
                yb["tile"] = yacc
            nc.scalar.activation(
                yb["tile"][:, k, :], out_ps, mybir.ActivationFunctionType.Tanh
            )
            if k == y_batch - 1 and y_stores:
                t0 = t - k
                st_eng.dma_start(out=yv[:, t0 : t0 + y_batch, :], in_=yb["tile"])

        def main_pipeline():
            if flow == "xw":
                # 3-stage software pipeline: matmuls(t) | norm-mul(t-1) |
                # stage2(t-2), so the PE never waits on the DVE multiply and
                # the DVE never waits on the PE broadcast of the same chunk.
                p1 = p2 = None
                for t in range(SPC * NB):
                    si, ib = divmod(t, NB)
                    if t < len(preloaded):
                        chunk_t = preloaded[t]
                    else:
                        chunk_t = load_chunk(si, ib)
                    if stages == "none":
                        continue
                    mT, deg = stage1_mm_xw(si, chunk_t)
                    if stages == "s1":
                        continue
                    if p1 is not None:
                        mts = norm_apply_xw(*p1[2:])
                        p2q = (p1[0], p1[1], mts)
                    else:
                        p2q = None
                    if p2 is not None:
                        stage2_xw(*p2)
                    p1 = (si, ib, mT, deg)
                    p2 = p2q
                if stages == "all":
                    mts = norm_apply_xw(*p1[2:])
                    if p2 is not None:
                        stage2_xw(*p2)
                    stage2_xw(p1[0], p1[1], mts)
                return
            prev = None
            for t in range(SPC * NB):
                si, ib = divmod(t, NB)
                if t < len(preloaded):
                    chunk_t = preloaded[t]
                else:
                    chunk_t = load_chunk(si, ib)
                if stages == "all":
                    ms = stage1_mm(si, chunk_t)
                    if prev is not None and pipe:
                        finish(*prev)
                    if pipe:
                        prev = (si, ib, *ms)
                    else:
                        finish(si, ib, *ms)
            if prev is not None:
                finish(*prev)

        if bench_iters > 1:
            # Bench mode: repeat the whole pipeline on-device so steady-state
            # HW time can be resolved through the ~88 ms axon dispatch noise.
            hints = (
                mybir.EngineType.PE,
                mybir.EngineType.DVE,
                mybir.EngineType.Activation,
                mybir.EngineType.Pool,
            )
            with tc.For_i(0, bench_iters, 1, hint_engines=hints):
                main_pipeline()
        else:
            main_pipeline()


_CACHE = {}


def build_nc(bench_iters=1, **knobs):
    nc = bacc.Bacc(
        "TRN2", target_bir_lowering=False, debug=False, num_devices=NCORES
    )
    with tile.TileContext(nc) as tc:
        _kernel_body(tc, bench_iters, **knobs)
    nc.compile()  # Bacc register-allocation / DCE pass
    return nc


def _get_nc():
    if "nc" not in _CACHE:
        _CACHE["nc"] = build_nc(1)
    return _CACHE["nc"]


A_DT = "f8"  # must match _kernel_body's a_dt default


def prep_A(A):
    """Host-side layout prep: drop the unused 5th relation channel, cast down
    (the baseline already did an equivalent lossy cast in-flight in the DMA),
    and permute to [s, ib, j(128), jb, c, i(128)] so tiles land lhsT-ready in
    SBUF with one contiguous run per (chunk, partition)."""
    import ml_dtypes

    npdt = ml_dtypes.float8_e4m3 if A_DT == "f8" else np.float16
    A4 = np.asarray(A).reshape(S, NB, 128, NB, 128, 5)[..., :4].astype(npdt)
    # [s, ib, i, jb, j, c] -> [s, ib, j, jb, c, i]
    return np.ascontiguousarray(A4.transpose(0, 1, 4, 3, 5, 2))


def shard_inputs(A_prepped, x, weight, theta_root, sl):
    import ml_dtypes

    xs = x[sl]
    return {
        "A": A_prepped[sl],
        "x8": np.ascontiguousarray(xs).astype(ml_dtypes.float8_e4m3),
        "xT": np.ascontiguousarray(xs.transpose(0, 2, 1).astype(np.float16)),
        "weight": weight,
        "theta_root": theta_root,
    }


LAST = None  # BassKernelResults of the most recent run (for profiling)


def kernel(A, x, weight, theta_root):
    global LAST
    x = np.ascontiguousarray(np.asarray(x), dtype=np.float32)
    weight = np.ascontiguousarray(np.asarray(weight), dtype=np.float32)
    theta_root = np.ascontiguousarray(np.asarray(theta_root), dtype=np.float32)
    At = prep_A(A)

    # The axon NTFF trace hook isn't shipped in this container; make sure a
    # stray BASS_TRACE=1 in the environment can't divert run_bass_kernel_spmd
    # into the (crashing) trace path.
    os.environ["BASS_NEVER_TRACE"] = "1"

    nc = _get_nc()
    in_maps = []
    for k in range(NCORES):
        sl = slice(k * SPC, (k + 1) * SPC)
        in_maps.append(shard_inputs(At, x, weight, theta_root, sl))
    res = run_bass_kernel_spmd(nc, in_maps, core_ids=list(range(NCORES)))
    LAST = res
    out = np.concatenate([r["y"] for r in res.results], axis=0)
    return np.ascontiguousarray(out.astype(np.float32))
